# revision 11
# baseline (speedup 1.0000x reference)
"""Trainium2 Bass kernel for nn_EdgePredictor (PointTransformer edge logits).

Row-parallel sharding across 8 NeuronCores: core c owns queries
[128c, 128c+128). Each layer launch computes, per core, the full
N/8 x N x d pairwise attention block on-device; the O(N d^2) per-layer
projections and the inter-layer feature gather are done host-side
between launches (collectives hang in this environment).

v4 (engine-balanced): per layer, per query i (1024 keys j, d=64,
attn hidden 256), with all per-pair [64, N] tensors chunk-packed
[128, 512] (keys split in two):
    h_ij  = relu(P1_i - P1_j + pb1) -> bf16             DVE (4x mode)
    u     = [pw2@aw1; -Wk@aw1].T @ [h; ftT] + b_i       PE bf16 K=128
    us    = relu(u) -> fp8                              ACT + DVE (stt)
    16sim = (16aw2).T @ us  (chunk0 fp8 DoubleRow)      PE
    e     = exp(sim), den += e (accum)                  ACT
    vv    = [pw2; Wv].T @ [h; ftT]                      PE bf16
    num  += sum_j e * vv  (scalar_tensor_tensor accum)  DVE
    out_i = num/den + pb2
ab2 is dropped: the per-dim softmax over j is invariant to it.
fp8/bf16 is safe: final logits min +54, sigmoid fully saturates
(numpy-emulated end-to-end rel err 0.0). DoubleRow outputs must start
at partition 0, so chunk 1 of sim uses regular fp8 matmuls. The Pool
engine is useless here (gpsimd can't touch PSUM; its tensor_scalar
runs ~14 ns/col).
"""
import numpy as np
import ml_dtypes

import concourse.bacc as bacc
import concourse.tile as tile
import concourse.mybir as mybir
from concourse.bass_utils import run_bass_kernel_spmd

F32 = mybir.dt.float32
BF16 = mybir.dt.bfloat16
FP8 = mybir.dt.float8e4
AF = mybir.ActivationFunctionType
ALU = mybir.AluOpType
PM = mybir.MatmulPerfMode

N = 1024
D = 64
NC = 8
OWN = N // NC  # 128 queries per core

TRACE = False          # test harness can flip this
LAST_EXEC_NS = []      # exec_time_ns of each launch when TRACE
DEBUG_FEATS = []       # per-layer feats (host view) for validation

_cache = {}
NQ = OWN  # queries emitted in the layer program (debug knob)

# cols of the usA evacuation handled by ACT (rest + all of usB go to DVE)
ACT_A1_COLS = 224


def _bf16(a):
    return np.ascontiguousarray(np.asarray(a, np.float32).astype(ml_dtypes.bfloat16))


def _f8(a):
    return np.ascontiguousarray(np.asarray(a, np.float32).astype(ml_dtypes.float8_e4m3fn))


def _f32(a):
    return np.ascontiguousarray(np.asarray(a, np.float32))


def build_layer_nc():
    """One attention layer for this core's 128 queries."""
    nc = bacc.Bacc("TRN2", target_bir_lowering=False, debug=False, num_devices=NC)
    d = {}
    for name, shape, dt in [
        ("negp1t", [D, N], BF16),    # -P1.T
        ("ftT", [D, N], BF16),       # feats.T (layer input)
        ("hbt", [D, OWN], F32),      # (P1_own + pb1).T per-i bias
        ("qaba", [128, OWN], F32),   # ((q+pb2)@aw1+ab1).T rows 0:128
        ("qabb", [128, OWN], F32),   # rows 128:256
        ("luA", [128, 128], BF16),   # [pw2@aw1; -Wk@aw1][:, 0:128]
        ("luB", [128, 128], BF16),   # [:, 128:256]
        ("lvv", [128, D], BF16),     # [pw2; Wv]
        ("la2a", [128, D], FP8),     # 16*aw2[0:128]
        ("la2b", [128, D], FP8),     # 16*aw2[128:256]
        ("sel", [128, D], F32),      # halves-add selector
        ("pb2col", [D, 1], F32),
    ]:
        d[name] = nc.dram_tensor(name, shape, dt, kind="ExternalInput")
    out_d = nc.dram_tensor("newown", [D, OWN], F32, kind="ExternalOutput")

    with tile.TileContext(nc) as tc:
        with (
            tc.tile_pool(name="cst", bufs=1) as cst,
            tc.tile_pool(name="hot", bufs=3) as hot,
            tc.tile_pool(name="psu", bufs=1, space="PSUM") as psu,
            tc.tile_pool(name="ps", bufs=2, space="PSUM") as ps,
        ):
            c = {}
            for name in ["negp1t", "hbt", "qaba", "qabb", "luA", "luB",
                         "lvv", "la2a", "la2b", "sel", "pb2col"]:
                t = cst.tile(list(d[name].shape), d[name].dtype, tag=name)
                nc.sync.dma_start(out=t[...], in_=d[name][...])
                c[name] = t
            HT = []
            for hi in range(3):
                Ht = cst.tile([128, N], BF16, tag=f"H{hi}")
                nc.sync.dma_start(out=Ht[64:128, :], in_=d["ftT"][:, :])
                HT.append(Ht)
            zer = cst.tile([128, 2, 512], BF16, tag="zer")
            nc.vector.memset(zer[...], 0.0)
            numb = cst.tile([128, OWN], F32, tag="numb")
            denb = cst.tile([128, OWN], F32, tag="denb")
            if NQ < OWN:  # debug: keep unwritten columns defined
                nc.vector.memset(numb[:, :], 1.0)
                nc.vector.memset(denb[:, :], 1.0)

            # Software-pipelined: per step, stage A (h/u/us/sim) runs for
            # query qA, exp runs one query behind (qE), vv+num two behind
            # (qB) so every engine's in-order queue stays stall-free.
            SA = ACT_A1_COLS
            e2q = {}
            simq = {}
            for it in range(NQ + 2):
                qA, qE, qB = it, it - 1, it - 2
                if 0 <= qE < NQ:
                    # e = exp(sim); den accumulates per query column  (ACT)
                    e2 = hot.tile([128, 512], BF16, tag="e2")
                    nc.scalar.activation(e2[:, :], simq.pop(qE)[:, :], AF.Exp,
                                         scale=1.0 / 16.0,
                                         accum_out=denb[:, qE:qE + 1])
                    e2q[qE] = e2
                if qA < NQ:
                    H = HT[qA % 3]
                    # h = relu(-P1T + (P1_i + pb1)) -> rows 0:64 (DVE 4x)
                    nc.vector.tensor_scalar(H[0:64, :], c["negp1t"][:, :],
                                            c["hbt"][:, qA:qA + 1], 0.0,
                                            ALU.add, ALU.max)
                if qB >= 0:
                    HB = HT[qB % 3]
                    # vv = pw2.T h + v_j, chunk-packed (bf16, K=128)  (PE)
                    vvp = ps.tile([128, 512], F32, tag="vvp")
                    for cc in range(2):
                        s = slice(512 * cc, 512 * (cc + 1))
                        nc.tensor.matmul(vvp[64 * cc:64 * cc + 64, :],
                                         c["lvv"][:, :], HB[:, s],
                                         start=True, stop=True)
                    # num += sum_j e * vv  (fused mult + reduce)  (DVE)
                    prs = hot.tile([128, 512], BF16, tag="prs")
                    nc.vector.scalar_tensor_tensor(
                        out=prs[:, :], in0=vvp[:, :], scalar=0.0,
                        in1=e2q.pop(qB)[:, :], op0=ALU.add, op1=ALU.mult,
                        accum_out=numb[:, qB:qB + 1])
                if qA < NQ:
                    # u quarters (bf16, K=128) -> four 1-bank PSUM tiles so
                    # the next iteration's matmul only waits on its own
                    # quarter's evacuation  (PE)
                    u00 = psu.tile([128, 512], F32, tag="u00")  # A chunk0
                    u01 = psu.tile([128, 512], F32, tag="u01")  # A chunk1
                    u10 = psu.tile([128, 512], F32, tag="u10")  # B chunk0
                    u11 = psu.tile([128, 512], F32, tag="u11")  # B chunk1
                    nc.tensor.matmul(u00[:, :], c["luA"][:, :], H[:, 0:512],
                                     start=True, stop=True)
                    nc.tensor.matmul(u10[:, :], c["luB"][:, :], H[:, 0:512],
                                     start=True, stop=True)
                    nc.tensor.matmul(u01[:, :], c["luA"][:, :], H[:, 512:1024],
                                     start=True, stop=True)
                    nc.tensor.matmul(u11[:, :], c["luB"][:, :], H[:, 512:1024],
                                     start=True, stop=True)
                    # us (fp8): one tile per evac instruction; a shared tile
                    # serializes its writers against each other
                    usa0 = hot.tile([128, 512], FP8, tag="usa0")  # ACT
                    usa1 = hot.tile([128, 512], FP8, tag="usa1")  # DVE
                    usb0 = hot.tile([128, 512], FP8, tag="usb0")  # DVE
                    usb1 = hot.tile([128, 512], FP8, tag="usb1")  # DVE
                    nc.scalar.activation(usa0[:, :], u00[:, :], AF.Relu,
                                         bias=c["qaba"][:, qA:qA + 1], scale=1.0)
                    nc.vector.scalar_tensor_tensor(
                        out=usb0[:, :], in0=u10[:, :],
                        scalar=c["qabb"][:, qA:qA + 1], in1=zer[:, 0, :],
                        op0=ALU.add, op1=ALU.max)
                    nc.vector.scalar_tensor_tensor(
                        out=usa1[:, :], in0=u01[:, :],
                        scalar=c["qaba"][:, qA:qA + 1], in1=zer[:, 0, :],
                        op0=ALU.add, op1=ALU.max)
                    nc.vector.scalar_tensor_tensor(
                        out=usb1[:, :], in0=u11[:, :],
                        scalar=c["qabb"][:, qA:qA + 1], in1=zer[:, 0, :],
                        op0=ALU.add, op1=ALU.max)
                    # 16*sim (fp8, K=256 via 2 accumulating matmuls/chunk)
                    simp = ps.tile([128, 512], F32, tag="simp")
                    nc.tensor.matmul(simp[0:64, :], c["la2a"][:, :],
                                     usa0[:, :], start=True, stop=False)
                    nc.tensor.matmul(simp[0:64, :], c["la2b"][:, :],
                                     usb0[:, :], start=False, stop=True)
                    nc.tensor.matmul(simp[64:128, :], c["la2a"][:, :],
                                     usa1[:, :], start=True, stop=False)
                    nc.tensor.matmul(simp[64:128, :], c["la2b"][:, :],
                                     usb1[:, :], start=False, stop=True)
                    simq[qA] = simp

            # combine chunk halves: sel.T @ [128, OWN] -> [64, OWN]
            ndp = ps.tile([D, OWN], F32, tag="simp")
            ddp = ps.tile([D, OWN], F32, tag="vvp")
            nc.tensor.matmul(ndp[:, :], c["sel"][:, :], numb[:, :],
                             start=True, stop=True)
            nc.tensor.matmul(ddp[:, :], c["sel"][:, :], denb[:, :],
                             start=True, stop=True)
            dds = cst.tile([D, OWN], F32, tag="dds")
            nc.vector.reciprocal(dds[:, :], ddp[:, :])
            div = cst.tile([D, OWN], F32, tag="div")
            now = cst.tile([D, OWN], F32, tag="now")
            nc.vector.tensor_tensor(out=div[:, :], in0=ndp[:, :], in1=dds[:, :],
                                    op=ALU.mult)
            nc.vector.tensor_scalar(now[:, :], div[:, :], c["pb2col"][:, :], None,
                                    ALU.add)
            nc.sync.dma_start(out=out_d[:, :], in_=now[:, :])
    nc.compile()
    return nc


def build_final_nc():
    """out_block = sigmoid(f1_own @ f1.T) [128, 1024] per core."""
    nc = bacc.Bacc("TRN2", target_bir_lowering=False, debug=False, num_devices=NC)
    f1t_d = nc.dram_tensor("f1t", [D, N], BF16, kind="ExternalInput")
    f1o_d = nc.dram_tensor("f1o", [D, OWN], BF16, kind="ExternalInput")
    out_d = nc.dram_tensor("blk", [OWN, N], F32, kind="ExternalOutput")
    with tile.TileContext(nc) as tc:
        with (
            tc.tile_pool(name="sb", bufs=1) as sb,
            tc.tile_pool(name="ps", bufs=2, space="PSUM") as ps,
        ):
            f1t = sb.tile([D, N], BF16, tag="f1t")
            f1o = sb.tile([D, OWN], BF16, tag="f1o")
            ot = sb.tile([OWN, N], F32, tag="ot")
            nc.sync.dma_start(out=f1t[:, :], in_=f1t_d[:, :])
            nc.sync.dma_start(out=f1o[:, :], in_=f1o_d[:, :])
            for chunk in range(2):
                s = slice(512 * chunk, 512 * (chunk + 1))
                op = ps.tile([OWN, 512], F32, tag="op")
                nc.tensor.matmul(op[:, :], f1o[:, :], f1t[:, s],
                                 start=True, stop=True)
                nc.scalar.activation(ot[:, s], op[:, :], AF.Sigmoid)
            nc.sync.dma_start(out=out_d[:, :], in_=ot[:, :])
    nc.compile()
    return nc


def _run(nc, in_maps):
    res = run_bass_kernel_spmd(nc, in_maps, list(range(NC)), trace=TRACE)
    if TRACE:
        LAST_EXEC_NS.append(res.exec_time_ns)
    return res.results


def kernel(x, in_w, in_b, qkv_w, pos_w1, pos_b1, pos_w2, pos_b2,
           attn_w1, attn_b1, attn_w2, attn_b2, fc_w, fc_b):
    x = np.asarray(x, np.float32)
    L = qkv_w.shape[0]
    if "layer" not in _cache:
        _cache["layer"] = build_layer_nc()
        _cache["final"] = build_final_nc()
    nc_layer, nc_final = _cache["layer"], _cache["final"]

    sel = np.zeros((128, D), np.float32)
    for p in range(128):
        sel[p, p % D] = 1.0

    feats = x @ _f32(in_w) + _f32(in_b)
    for l in range(L):
        qkv = feats @ _f32(qkv_w[l])
        q = qkv[:, :D]
        P1 = x @ _f32(pos_w1[l][:2])  # pos z == 0
        pw2 = _f32(pos_w2[l])
        aw1 = _f32(attn_w1[l])
        aw2 = _f32(attn_w2[l])
        Wk = _f32(qkv_w[l][:, D:2 * D])
        Wv = _f32(qkv_w[l][:, 2 * D:])
        Btot = np.concatenate([pw2 @ aw1, -(Wk @ aw1)], 0)   # [128, 256]
        qab = (q + _f32(pos_b2[l])) @ aw1 + _f32(attn_b1[l])
        in_maps = []
        for cix in range(NC):
            own = slice(OWN * cix, OWN * (cix + 1))
            in_maps.append({
                "negp1t": _bf16(-P1.T),
                "ftT": _bf16(feats.T),
                "hbt": _f32((P1[own] + _f32(pos_b1[l])).T),
                "qaba": _f32(qab[own, 0:128].T),
                "qabb": _f32(qab[own, 128:256].T),
                "luA": _bf16(Btot[:, 0:128]),
                "luB": _bf16(Btot[:, 128:256]),
                "lvv": _bf16(np.concatenate([pw2, Wv], 0)),
                "la2a": _f8(16.0 * aw2[:128]),
                "la2b": _f8(16.0 * aw2[128:]),
                "sel": sel,
                "pb2col": _f32(pos_b2[l])[:, None],
            })
        results = _run(nc_layer, in_maps)
        feats = np.concatenate([results[cix]["newown"].T for cix in range(NC)], 0)
        DEBUG_FEATS.append(feats)

    f1 = feats @ _f32(fc_w) + _f32(fc_b)
    f1T = _bf16(f1.T)
    in_maps = [{"f1t": f1T,
                "f1o": _bf16(f1[OWN * cix:OWN * (cix + 1)].T)}
               for cix in range(NC)]
    results = _run(nc_final, in_maps)
    return np.concatenate([results[cix]["blk"] for cix in range(NC)], 0)


# revision 15
# speedup vs baseline: 1.1688x; 1.1688x over previous
"""Trainium2 Bass kernel for nn_EdgePredictor (PointTransformer edge logits).

Row-parallel sharding across 8 NeuronCores: core c owns queries
[128c, 128c+128). Each layer launch computes, per core, the full
N/8 x N x d pairwise attention block on-device; the O(N d^2) per-layer
projections and the inter-layer feature gather are done host-side
between launches (collectives hang in this environment).

v4 (engine-balanced): per layer, per query i (1024 keys j, d=64,
attn hidden 256), with all per-pair [64, N] tensors chunk-packed
[128, 512] (keys split in two):
    h_ij  = relu(P1_i - P1_j + pb1) -> bf16             DVE (4x mode)
    u     = [pw2@aw1; -Wk@aw1].T @ [h; ftT] + b_i       PE bf16 K=128
    us    = relu(u) -> fp8                              ACT + DVE (stt)
    16sim = (16aw2).T @ us  (chunk0 fp8 DoubleRow)      PE
    e     = exp(sim), den += e (accum)                  ACT
    vv    = [pw2; Wv].T @ [h; ftT]                      PE bf16
    num  += sum_j e * vv  (scalar_tensor_tensor accum)  DVE
    out_i = num/den + pb2
ab2 is dropped: the per-dim softmax over j is invariant to it.
fp8/bf16 is safe: final logits min +54, sigmoid fully saturates
(numpy-emulated end-to-end rel err 0.0). DoubleRow outputs must start
at partition 0, so chunk 1 of sim uses regular fp8 matmuls. The Pool
engine is useless here (gpsimd can't touch PSUM; its tensor_scalar
runs ~14 ns/col).
"""
import numpy as np
import ml_dtypes

import concourse.bacc as bacc
import concourse.tile as tile
import concourse.mybir as mybir
from concourse.bass_utils import run_bass_kernel_spmd

F32 = mybir.dt.float32
BF16 = mybir.dt.bfloat16
FP8 = mybir.dt.float8e4
AF = mybir.ActivationFunctionType
ALU = mybir.AluOpType
PM = mybir.MatmulPerfMode

N = 1024
D = 64
NC = 8
OWN = N // NC  # 128 queries per core

TRACE = False          # test harness can flip this
LAST_EXEC_NS = []      # exec_time_ns of each launch when TRACE
DEBUG_FEATS = []       # per-layer feats (host view) for validation

_cache = {}
NQ = OWN  # queries emitted in the layer program (debug knob)

# cols of the usA evacuation handled by ACT (rest + all of usB go to DVE)
ACT_A1_COLS = 224


def _bf16(a):
    return np.ascontiguousarray(np.asarray(a, np.float32).astype(ml_dtypes.bfloat16))


def _f8(a):
    return np.ascontiguousarray(np.asarray(a, np.float32).astype(ml_dtypes.float8_e4m3fn))


def _f32(a):
    return np.ascontiguousarray(np.asarray(a, np.float32))


def build_layer_nc():
    """One attention layer for this core's 128 queries."""
    nc = bacc.Bacc("TRN2", target_bir_lowering=False, debug=False, num_devices=NC)
    d = {}
    for name, shape, dt in [
        ("negp1t", [D, N], BF16),    # -P1.T
        ("ftT", [D, N], BF16),       # feats.T (layer input)
        ("hbt", [D, OWN], F32),      # (P1_own + pb1).T per-i bias
        ("qaba", [128, OWN], F32),   # ((q+pb2)@aw1+ab1).T rows 0:128
        ("qabb", [128, OWN], F32),   # rows 128:256
        ("luA", [128, 128], BF16),   # [pw2@aw1; -Wk@aw1][:, 0:128]
        ("luB", [128, 128], BF16),   # [:, 128:256]
        ("lvv", [128, D], BF16),     # [pw2; Wv]
        ("la2a", [128, D], FP8),     # 16*aw2[0:128]
        ("la2b", [128, D], FP8),     # 16*aw2[128:256]
        ("la2dr", [128, 2, D], FP8), # 16*aw2 k-tiled (DoubleRow interleave)
        ("sel", [128, D], F32),      # halves-add selector
        ("pb2col", [D, 1], F32),
    ]:
        d[name] = nc.dram_tensor(name, shape, dt, kind="ExternalInput")
    out_d = nc.dram_tensor("newown", [D, OWN], F32, kind="ExternalOutput")

    with tile.TileContext(nc) as tc:
        with (
            tc.tile_pool(name="cst", bufs=1) as cst,
            tc.tile_pool(name="hot", bufs=3) as hot,
            tc.tile_pool(name="psu", bufs=1, space="PSUM") as psu,
            tc.tile_pool(name="ps", bufs=2, space="PSUM") as ps,
        ):
            c = {}
            for name in ["negp1t", "hbt", "qaba", "qabb", "luA", "luB",
                         "lvv", "la2a", "la2b", "la2dr", "sel", "pb2col"]:
                t = cst.tile(list(d[name].shape), d[name].dtype, tag=name)
                nc.sync.dma_start(out=t[...], in_=d[name][...])
                c[name] = t
            HT = []
            for hi in range(3):
                Ht = cst.tile([128, N], BF16, tag=f"H{hi}")
                nc.sync.dma_start(out=Ht[64:128, :], in_=d["ftT"][:, :])
                HT.append(Ht)
            zer = cst.tile([128, 2, 512], BF16, tag="zer")
            nc.vector.memset(zer[...], 0.0)
            numb = cst.tile([128, OWN], F32, tag="numb")
            denb = cst.tile([128, OWN], F32, tag="denb")
            if NQ < OWN:  # debug: keep unwritten columns defined
                nc.vector.memset(numb[:, :], 1.0)
                nc.vector.memset(denb[:, :], 1.0)

            # Software-pipelined: per step, stage A (h/u/us/sim) runs for
            # query qA, exp runs one query behind (qE), vv+num two behind
            # (qB) so every engine's in-order queue stays stall-free.
            SA = ACT_A1_COLS
            e2q = {}
            simq = {}
            for it in range(NQ + 2):
                qA, qE, qB = it, it - 1, it - 2
                if 0 <= qE < NQ:
                    # e = exp(sim); den accumulates per query column  (ACT)
                    e2 = hot.tile([128, 512], BF16, tag="e2")
                    nc.scalar.activation(e2[:, :], simq.pop(qE)[:, :], AF.Exp,
                                         scale=1.0 / 16.0,
                                         accum_out=denb[:, qE:qE + 1])
                    e2q[qE] = e2
                if qA < NQ:
                    H = HT[qA % 3]
                    # h = relu(-P1T + (P1_i + pb1)) -> rows 0:64 (DVE 4x)
                    nc.vector.tensor_scalar(H[0:64, :], c["negp1t"][:, :],
                                            c["hbt"][:, qA:qA + 1], 0.0,
                                            ALU.add, ALU.max)
                if qB >= 0:
                    HB = HT[qB % 3]
                    # vv = pw2.T h + v_j, chunk-packed (bf16, K=128)  (PE)
                    vvp = ps.tile([128, 512], F32, tag="vvp")
                    for cc in range(2):
                        s = slice(512 * cc, 512 * (cc + 1))
                        nc.tensor.matmul(vvp[64 * cc:64 * cc + 64, :],
                                         c["lvv"][:, :], HB[:, s],
                                         start=True, stop=True)
                if qA < NQ:
                    # u quarters (bf16, K=128) -> four 1-bank PSUM tiles so
                    # the next iteration's matmul only waits on its own
                    # quarter's evacuation  (PE)
                    u00 = psu.tile([128, 512], F32, tag="u00")  # A chunk0
                    u01 = psu.tile([128, 512], F32, tag="u01")  # A chunk1
                    u10 = psu.tile([128, 512], F32, tag="u10")  # B chunk0
                    u11 = psu.tile([128, 512], F32, tag="u11")  # B chunk1
                    nc.tensor.matmul(u00[:, :], c["luA"][:, :], H[:, 0:512],
                                     start=True, stop=True)
                    nc.tensor.matmul(u10[:, :], c["luB"][:, :], H[:, 0:512],
                                     start=True, stop=True)
                    nc.tensor.matmul(u01[:, :], c["luA"][:, :], H[:, 512:1024],
                                     start=True, stop=True)
                    nc.tensor.matmul(u11[:, :], c["luB"][:, :], H[:, 512:1024],
                                     start=True, stop=True)
                    # us (fp8): chunk0 goes interleaved into us0 (both halves
                    # written by ACT -> usable as DoubleRow rhs), chunk1 into
                    # us1 (both halves by DVE). num is issued after the DVE
                    # evacs so sim never waits behind it.
                    us0 = hot.tile([128, 2, 512], FP8, tag="us0")  # ACT
                    us1 = hot.tile([128, 2, 512], FP8, tag="us1")  # DVE
                    nc.scalar.activation(us0[:, 0, :], u00[:, :], AF.Relu,
                                         bias=c["qaba"][:, qA:qA + 1], scale=1.0)
                    nc.scalar.activation(us0[:, 1, :], u10[:, :], AF.Relu,
                                         bias=c["qabb"][:, qA:qA + 1], scale=1.0)
                    nc.vector.scalar_tensor_tensor(
                        out=us1[:, 0, :], in0=u01[:, :],
                        scalar=c["qaba"][:, qA:qA + 1], in1=zer[:, 0, :],
                        op0=ALU.add, op1=ALU.max)
                    nc.vector.scalar_tensor_tensor(
                        out=us1[:, 1, :], in0=u11[:, :],
                        scalar=c["qabb"][:, qA:qA + 1], in1=zer[:, 0, :],
                        op0=ALU.add, op1=ALU.max)
                    # 16*sim: chunk1 regular fp8 accum (ready first), chunk0
                    # DoubleRow on the ACT-written interleaved tile
                    simp = ps.tile([128, 512], F32, tag="simp")
                    nc.tensor.matmul(simp[64:128, :], c["la2a"][:, :],
                                     us1[:, 0, :], start=True, stop=False)
                    nc.tensor.matmul(simp[64:128, :], c["la2b"][:, :],
                                     us1[:, 1, :], start=False, stop=True)
                    nc.tensor.matmul(simp[0:64, :], c["la2dr"][...],
                                     us0[:, :, :], start=True, stop=True,
                                     perf_mode=PM.DoubleRow)
                    simq[qA] = simp
                if qB >= 0:
                    # num += sum_j e * vv  (fused mult + reduce)  (DVE, last)
                    prs = hot.tile([128, 512], BF16, tag="prs")
                    nc.vector.scalar_tensor_tensor(
                        out=prs[:, :], in0=vvp[:, :], scalar=0.0,
                        in1=e2q.pop(qB)[:, :], op0=ALU.add, op1=ALU.mult,
                        accum_out=numb[:, qB:qB + 1])

            # combine chunk halves: sel.T @ [128, OWN] -> [64, OWN]
            ndp = ps.tile([D, OWN], F32, tag="simp")
            ddp = ps.tile([D, OWN], F32, tag="vvp")
            nc.tensor.matmul(ndp[:, :], c["sel"][:, :], numb[:, :],
                             start=True, stop=True)
            nc.tensor.matmul(ddp[:, :], c["sel"][:, :], denb[:, :],
                             start=True, stop=True)
            dds = cst.tile([D, OWN], F32, tag="dds")
            nc.vector.reciprocal(dds[:, :], ddp[:, :])
            div = cst.tile([D, OWN], F32, tag="div")
            now = cst.tile([D, OWN], F32, tag="now")
            nc.vector.tensor_tensor(out=div[:, :], in0=ndp[:, :], in1=dds[:, :],
                                    op=ALU.mult)
            nc.vector.tensor_scalar(now[:, :], div[:, :], c["pb2col"][:, :], None,
                                    ALU.add)
            nc.sync.dma_start(out=out_d[:, :], in_=now[:, :])
    nc.compile()
    return nc


def build_final_nc():
    """out_block = sigmoid(f1_own @ f1.T) [128, 1024] per core."""
    nc = bacc.Bacc("TRN2", target_bir_lowering=False, debug=False, num_devices=NC)
    f1t_d = nc.dram_tensor("f1t", [D, N], BF16, kind="ExternalInput")
    f1o_d = nc.dram_tensor("f1o", [D, OWN], BF16, kind="ExternalInput")
    out_d = nc.dram_tensor("blk", [OWN, N], F32, kind="ExternalOutput")
    with tile.TileContext(nc) as tc:
        with (
            tc.tile_pool(name="sb", bufs=1) as sb,
            tc.tile_pool(name="ps", bufs=2, space="PSUM") as ps,
        ):
            f1t = sb.tile([D, N], BF16, tag="f1t")
            f1o = sb.tile([D, OWN], BF16, tag="f1o")
            ot = sb.tile([OWN, N], F32, tag="ot")
            nc.sync.dma_start(out=f1t[:, :], in_=f1t_d[:, :])
            nc.sync.dma_start(out=f1o[:, :], in_=f1o_d[:, :])
            for chunk in range(2):
                s = slice(512 * chunk, 512 * (chunk + 1))
                op = ps.tile([OWN, 512], F32, tag="op")
                nc.tensor.matmul(op[:, :], f1o[:, :], f1t[:, s],
                                 start=True, stop=True)
                nc.scalar.activation(ot[:, s], op[:, :], AF.Sigmoid)
            nc.sync.dma_start(out=out_d[:, :], in_=ot[:, :])
    nc.compile()
    return nc


def _run(nc, in_maps):
    res = run_bass_kernel_spmd(nc, in_maps, list(range(NC)), trace=TRACE)
    if TRACE:
        LAST_EXEC_NS.append(res.exec_time_ns)
    return res.results


def kernel(x, in_w, in_b, qkv_w, pos_w1, pos_b1, pos_w2, pos_b2,
           attn_w1, attn_b1, attn_w2, attn_b2, fc_w, fc_b):
    x = np.asarray(x, np.float32)
    L = qkv_w.shape[0]
    if "layer" not in _cache:
        _cache["layer"] = build_layer_nc()
        _cache["final"] = build_final_nc()
    nc_layer, nc_final = _cache["layer"], _cache["final"]

    sel = np.zeros((128, D), np.float32)
    for p in range(128):
        sel[p, p % D] = 1.0

    feats = x @ _f32(in_w) + _f32(in_b)
    for l in range(L):
        qkv = feats @ _f32(qkv_w[l])
        q = qkv[:, :D]
        P1 = x @ _f32(pos_w1[l][:2])  # pos z == 0
        pw2 = _f32(pos_w2[l])
        aw1 = _f32(attn_w1[l])
        aw2 = _f32(attn_w2[l])
        Wk = _f32(qkv_w[l][:, D:2 * D])
        Wv = _f32(qkv_w[l][:, 2 * D:])
        Btot = np.concatenate([pw2 @ aw1, -(Wk @ aw1)], 0)   # [128, 256]
        qab = (q + _f32(pos_b2[l])) @ aw1 + _f32(attn_b1[l])
        in_maps = []
        for cix in range(NC):
            own = slice(OWN * cix, OWN * (cix + 1))
            in_maps.append({
                "negp1t": _bf16(-P1.T),
                "ftT": _bf16(feats.T),
                "hbt": _f32((P1[own] + _f32(pos_b1[l])).T),
                "qaba": _f32(qab[own, 0:128].T),
                "qabb": _f32(qab[own, 128:256].T),
                "luA": _bf16(Btot[:, 0:128]),
                "luB": _bf16(Btot[:, 128:256]),
                "lvv": _bf16(np.concatenate([pw2, Wv], 0)),
                "la2a": _f8(16.0 * aw2[:128]),
                "la2b": _f8(16.0 * aw2[128:]),
                "la2dr": _f8(np.stack([16.0 * aw2[:128], 16.0 * aw2[128:]],
                                      axis=1)),
                "sel": sel,
                "pb2col": _f32(pos_b2[l])[:, None],
            })
        results = _run(nc_layer, in_maps)
        feats = np.concatenate([results[cix]["newown"].T for cix in range(NC)], 0)
        DEBUG_FEATS.append(feats)

    f1 = feats @ _f32(fc_w) + _f32(fc_b)
    f1T = _bf16(f1.T)
    in_maps = [{"f1t": f1T,
                "f1o": _bf16(f1[OWN * cix:OWN * (cix + 1)].T)}
               for cix in range(NC)]
    results = _run(nc_final, in_maps)
    return np.concatenate([results[cix]["blk"] for cix in range(NC)], 0)
